# revision 22
# baseline (speedup 1.0000x reference)
"""Trainium2 Bass kernel: single-head causal attention (B=8, T=2048, D=1024, HS=64).

Sharding: data-parallel over batch B -- one batch element per NeuronCore (8 cores).
Host-side prep (part of sharding/layout): per-core x is passed transposed (d-major)
so the contraction dim lands on SBUF partitions; weights are packed host-side into
the exact SBUF layout so every DMA is contiguous (hardware DGE, few descriptors).

Per-core device algorithm (bf16 matmul mode, f32 PSUM accumulation):
  x.T streams in per 512-query column block (j-major) on both HWDGE queues, so
  projections of block 0 start as soon as its 1MB lands instead of after the
  full 4MB.
  [Q.T; K.T] (stacked on partitions) = [wq; wk].T-chunks @ x.T (PSUM-accumulated,
  cols 0:512 of a 2-bank PSUM tile); V.T = wv.T-chunks @ x.T (cols 512:1024).
  K bias is dropped entirely: S[tq,tk] += q.bk is constant per query row, so
  softmax is invariant to it.  Q/V biases are fused into the PSUM->SBUF moves.
  V is naturalized [t, h] via the DMA XBAR transpose (zero PE/DVE cost) with an
  appended ones-column (row-sum trick for the softmax denominator).
  Attention in transposed layout: S.T[tk, tq] = K.T_chunk.T @ Q.T, with key
  chunks processed in PAIRS sharing one 2-bank PSUM tile so one ACT exp covers
  1024 columns (scale 1/sqrt(HS) fused; no max-subtraction -- scores are O(1)
  gaussian).  Causal: chunk skipping, triangular moving-range slicing, and a
  0/1 triangular bf16 mask multiply on DVE for diagonal chunks.
  S-pair / PV-pair matmuls are software-pipelined one pair deep so the PE never
  waits on the ACT exp.  O.T_unnorm[h+1, tq] accumulates V'_chunk.T @ P.T; row
  HS is the denominator.  Final PE transpose to [tq, h+1], DVE reciprocal+mul,
  contiguous DMA out; the host reassembles the block layout.
"""
import os
import sys

for _p in ("/opt/trn_rl_repo", "/root/.axon_site/_ro/trn_rl_repo"):
    if _p not in sys.path and os.path.isdir(_p):
        sys.path.append(_p)

import numpy as np
import jax

try:
    jax.config.update("jax_compilation_cache_dir", "/tmp/jax_neff_cache")
    jax.config.update("jax_persistent_cache_min_compile_time_secs", 1.0)
    jax.config.update("jax_persistent_cache_min_entry_size_bytes", -1)
except Exception:
    pass

import concourse.mybir as mybir
import concourse.tile as tile
from concourse import bacc
from concourse.bass_utils import run_bass_kernel_spmd
from concourse.masks import make_identity

B, T, D, HS = 8, 2048, 1024, 64
NCORES = 8
QB = 512            # query block (free dim of S.T tiles / PSUM bank width)
KC = 128            # key chunk (partition dim of S.T tiles)
NQB = T // QB       # 4
NKC = T // KC       # 16
ND = D // 128       # 8 contraction chunks

MM_MODE = os.environ.get("BASS_MM_MODE", "bf16")   # "bf16" | "f32"
FALLBACK_MODE = "f32"   # numerically safe mode if the fast mode misbehaves on HW

F32 = mybir.dt.float32
_MM_DTS = {"f32": F32, "f32r": mybir.dt.float32r, "bf16": mybir.dt.bfloat16}


def build(mode=None):
    MM = _MM_DTS[mode or MM_MODE]
    xbar_ok = (MM is mybir.dt.bfloat16)   # DMA XBAR transpose needs 2-byte dtype
    nc = bacc.Bacc(None)
    xT = nc.declare_dram_parameter("xT", [D, T], MM, isOutput=False)
    wqkB = nc.declare_dram_parameter("wqkB", [128, ND, 2 * HS], MM, isOutput=False)
    wvB = nc.declare_dram_parameter("wvB", [128, ND, HS], MM, isOutput=False)
    qb = nc.declare_dram_parameter("qb", [HS, 1], F32, isOutput=False)
    vb = nc.declare_dram_parameter("vb", [HS, 1], F32, isOutput=False)
    # out[p, j*4+tt, h] = attention output for query t = j*512 + tt*128 + p
    out = nc.declare_dram_parameter("out", [128, NQB * 4, HS], F32, isOutput=True)

    scale = float(1.0 / np.sqrt(HS))

    with tile.TileContext(nc) as tc:
        with tc.tile_pool(name="const", bufs=1) as cpool, \
             tc.tile_pool(name="big", bufs=1) as bpool, \
             tc.tile_pool(name="pex", bufs=5) as ppool, \
             tc.tile_pool(name="fin", bufs=2) as fpool, \
             tc.tile_pool(name="psS", bufs=3, space="PSUM") as psS, \
             tc.tile_pool(name="psO", bufs=1, space="PSUM") as psO, \
             tc.tile_pool(name="psT", bufs=1, space="PSUM") as psT:

            # ---- weights/bias loads first (contiguous, host-packed) ----
            wqk_t = cpool.tile([128, ND, 2 * HS], MM, tag="wqk")
            nc.scalar.dma_start(wqk_t[:], wqkB[:])
            wv_t = cpool.tile([128, ND, HS], MM, tag="wv")
            nc.scalar.dma_start(wv_t[:], wvB[:])
            qb_t = cpool.tile([HS, 1], F32, tag="qb")
            nc.scalar.dma_start(qb_t[:], qb[:])
            vb_t = cpool.tile([HS, 1], F32, tag="vb")
            nc.scalar.dma_start(vb_t[:], vb[:])

            # ---- x.T streaming loads: column HALVES (2KB lines) ----
            # DMA trigger cost ~ descriptor count (1/partition-line) and
            # per-queue BW ~ line size, so fewer/wider lines win.  Half 0
            # feeds blocks 0-1, half 1 feeds blocks 2-3; both HWDGE queues.
            xTs = bpool.tile([128, ND, T], MM, tag="xTs")
            for h in range(2):
                csl = slice(h * T // 2, (h + 1) * T // 2)
                for dc in range(ND):
                    if h == 0 and dc >= 6:
                        # 3rd (software) channel: slow to start, so it gets
                        # the chunks the dc accumulation loop consumes last
                        eng = nc.gpsimd
                    else:
                        eng = nc.sync if (dc % 2 == 0) else nc.scalar
                    eng.dma_start(xTs[:, dc, csl],
                                  xT[dc * 128:(dc + 1) * 128, csl])

            # ---- constants: identity FIRST so PE warmup starts early ----
            id_32 = cpool.tile([128, 128], F32, tag="id_32")
            make_identity(nc, id_32[:])
            if MM is F32:
                id_mm = id_32
            else:
                id_mm = cpool.tile([128, 128], MM, tag="id_mm")
                nc.vector.tensor_copy(id_mm[:], id_32[:])

            # warm the PE (HAM clock gate) with throwaway transposes while the
            # first x.T half lands; keep it short so proj0 is not delayed
            wu = psT.tile([128, 128], MM, tag="pt2")
            for _ in range(9):
                nc.tensor.transpose(wu[:], id_mm[:], id_mm[:])

            # 0/1 lower-causal mask for diagonal S.T chunks (keep iff f >= p)
            trimask_f = cpool.tile([128, QB], F32, tag="trimask_f")
            nc.gpsimd.memset(trimask_f[:], 1.0)
            nc.gpsimd.affine_select(
                out=trimask_f[:], in_=trimask_f[:],
                compare_op=mybir.AluOpType.is_ge,
                fill=0.0, base=0,
                pattern=[[1, QB]], channel_multiplier=-1)
            if MM is F32:
                trimask = trimask_f
            else:
                trimask = cpool.tile([128, QB], MM, tag="trimask")
                nc.vector.tensor_copy(trimask[:], trimask_f[:])

            # V in natural layout [t, h] + ones column (denominator row-sum)
            # last dim padded to HS+2 so 2-byte chunk offsets stay 4B-aligned
            Vn = bpool.tile([128, NKC, HS + 2], MM, tag="Vn")
            ones16 = cpool.tile([128, NKC, 1], F32, tag="ones16")
            nc.gpsimd.memset(ones16[:], 1.0)
            nc.vector.tensor_copy(Vn[:, :, HS:HS + 1], ones16[:])

            QT = bpool.tile([64, T], MM, tag="QT")
            KT = bpool.tile([64, T], MM, tag="KT")
            VTr = bpool.tile([64, T], MM, tag="VTr")

            def finalize(j, po):
                # transpose [65, 512] -> [128, 4, 65], normalize by row HS,
                # contiguous store (host reassembles the block layout).
                # Blocks 0-2 transpose on the DMA XBAR (frees PE rows); the
                # last block keeps the PE path so no XBAR latency lands in
                # the kernel tail.  (XBAR needs 2-byte dtype + 16|p_dim.)
                use_xbar = xbar_ok and j < NQB - 1
                if use_xbar:
                    # 80 partitions: pad 65 up to the 16-row XBAR tile grid;
                    # rows 65:80 are never read back (out cols 65:80 unused)
                    ob = fpool.tile([80, QB], MM, tag="ob", padded_shape=None)
                    nc.vector.tensor_copy(ob[0:HS + 1, :], po[0:HS + 1, :])
                    pt2s = fpool.tile([128, 4, 80], MM, tag="pt2s")
                    nc.sync.dma_start_transpose(pt2s[:], ob[:])
                    src_t = pt2s
                else:
                    ob = fpool.tile([80, QB], MM, tag="ob")
                    # two half copies: transposes of tt 0-1 overlap the copy
                    # of the second half
                    nc.vector.tensor_copy(ob[0:HS + 1, 0:QB // 2],
                                          po[0:HS + 1, 0:QB // 2])
                    nc.vector.tensor_copy(ob[0:HS + 1, QB // 2:QB],
                                          po[0:HS + 1, QB // 2:QB])
                    pt2 = psT.tile([128, 4, HS + 2], MM if xbar_ok else F32,
                                   tag="pt2")
                    idt = (id_mm if xbar_ok else id_32)[0:HS + 1, 0:HS + 1]
                    for tt in range(4):
                        nc.tensor.transpose(pt2[:, tt, 0:HS + 1],
                                            ob[:HS + 1, tt * 128:(tt + 1) * 128],
                                            idt)
                    src_t = pt2
                rc = fpool.tile([128, 4], F32, tag="rc")
                fin = fpool.tile([128, 4, HS], F32, tag="fin")
                if j == NQB - 1:
                    # tail-critical block: pipeline per half -- recip/scale/
                    # store of tt 0-1 overlap the transposes of tt 2-3
                    for hh in range(2):
                        t0 = 2 * hh
                        nc.vector.reciprocal(rc[:, t0:t0 + 2],
                                             src_t[:, t0:t0 + 2, HS])
                        nc.vector.tensor_scalar_mul(fin[:, t0, :],
                                                    src_t[:, t0, 0:HS],
                                                    rc[:, t0:t0 + 1])
                        nc.scalar.activation(fin[:, t0 + 1, :],
                                             src_t[:, t0 + 1, 0:HS],
                                             mybir.ActivationFunctionType.Copy,
                                             scale=rc[:, t0 + 1:t0 + 2])
                        nc.sync.dma_start(
                            out[:, 4 * j + t0:4 * j + t0 + 2, :],
                            fin[:, t0:t0 + 2, :])
                else:
                    nc.vector.reciprocal(rc[:], src_t[:, :, HS])
                    for tt in range(4):
                        nc.vector.tensor_scalar_mul(fin[:, tt, :],
                                                    src_t[:, tt, 0:HS],
                                                    rc[:, tt:tt + 1])
                    nc.sync.dma_start(out[:, 4 * j:4 * j + 4, :], fin[:])

            def proj_moves(j, ps):
                # PSUM -> SBUF with fused biases; K bias dropped
                # (softmax-invariant) so its move rides the ACT engine
                sl = slice(j * QB, (j + 1) * QB)
                nc.vector.tensor_scalar_add(QT[:, sl], ps[0:HS, 0:QB], qb_t[:])
                nc.scalar.activation(KT[:, sl], ps[HS:128, 0:QB],
                                     mybir.ActivationFunctionType.Copy)
                nc.vector.tensor_scalar_add(VTr[:, sl], ps[0:HS, QB:2 * QB],
                                            vb_t[:])
                # naturalize V chunks 4j..4j+3: [64, 512] -> [128, 4, 64]
                if xbar_ok:
                    # XBAR writes a contiguous tile; a cheap bf16 2x DVE copy
                    # threads it into Vn's padded layout
                    vt4 = fpool.tile([128, 4, HS], MM, tag="vt4")
                    nc.sync.dma_start_transpose(vt4[:], VTr[:, sl])
                    nc.vector.tensor_copy(Vn[:, 4 * j:4 * j + 4, 0:HS], vt4[:])
                else:
                    for c in range(4 * j, 4 * j + 4):
                        ptv = psT.tile([128, 4, HS + 2], F32, tag="pt2")
                        nc.tensor.transpose(ptv[:, 0, 0:HS],
                                            VTr[:, c * 128:(c + 1) * 128],
                                            id_mm[0:HS, 0:HS])
                        nc.vector.tensor_copy(Vn[:, c, 0:HS], ptv[:, 0, 0:HS])

            def proj_mms(j, ps, dc):
                sl = slice(j * QB, (j + 1) * QB)
                nc.tensor.matmul(ps[:, 0:QB], wqk_t[:, dc, :], xTs[:, dc, sl],
                                 start=(dc == 0), stop=(dc == ND - 1))
                nc.tensor.matmul(ps[0:HS, QB:2 * QB], wv_t[:, dc, :],
                                 xTs[:, dc, sl],
                                 start=(dc == 0), stop=(dc == ND - 1))

            def attn(j):
                ncl = 4 * j + 4          # causal chunk count
                P = ncl // 2             # key-chunk pairs
                # first diagonal pair leads (its chunk 4j resets the po
                # bank), nondiagonal pairs hide the masks' latency, and the
                # second diagonal pair closes the block: its last chunk is
                # only 128 queries wide, so the block tail is short
                order = [2 * j] + list(range(2 * j)) + [2 * j + 1]
                seq = []                 # PV emission order = order
                po = psO.tile([128, QB], F32, tag="opsum")
                for i, p in enumerate(order):
                    st = psS.tile([128, 2 * QB], F32, tag="spair")
                    pe = ppool.tile([128, 2 * QB], MM, tag="pexp")
                    info = []
                    for half, c in enumerate((2 * p, 2 * p + 1)):
                        f0 = max(0, 128 * (c - 4 * j))
                        off = half * QB
                        qsl = slice(j * QB + f0, (j + 1) * QB)
                        nc.tensor.matmul(st[:, off + f0:off + QB],
                                         KT[:, c * 128:(c + 1) * 128],
                                         QT[:, qsl], start=True, stop=True)
                        info.append((c, f0, off))
                    if info[0][1] == 0 and info[1][1] == 0:
                        # both chunks full width: one exp over both banks
                        nc.scalar.activation(pe[:], st[:],
                                             mybir.ActivationFunctionType.Exp,
                                             scale=scale)
                    else:
                        for c, f0, off in info:
                            nc.scalar.activation(
                                pe[:, off + f0:off + QB], st[:, off + f0:off + QB],
                                mybir.ActivationFunctionType.Exp, scale=scale)
                    for c, f0, off in info:
                        if c >= 4 * j:
                            # keep S.T[p, f0+f'] iff f' >= p  (tk <= tq)
                            nc.vector.tensor_mul(pe[:, off + f0:off + QB],
                                                 pe[:, off + f0:off + QB],
                                                 trimask[:, 0:QB - f0])
                    seq.append((pe, info))
                    if i >= 1:
                        _pv(nc, po, Vn, seq[i - 1], i - 1, len(order))
                _pv(nc, po, Vn, seq[P - 1], P - 1, len(order))
                return po

            # blocks 0/1 projections dc-interleaved: each chunk's matmuls run
            # as its half-0 slice lands, keeping the PE busy during the load
            ps0 = psS.tile([128, 2 * QB], F32, tag="spair")
            ps1 = psS.tile([128, 2 * QB], F32, tag="spair")
            for dc in range(ND):
                proj_mms(0, ps0, dc)
                proj_mms(1, ps1, dc)
            proj_moves(0, ps0)
            proj_moves(1, ps1)
            po0 = attn(0)
            finalize(0, po0)
            po1 = attn(1)
            for j in (2, 3):
                ps = psS.tile([128, 2 * QB], F32, tag="spair")
                for dc in range(ND):
                    proj_mms(j, ps, dc)
                finalize(j - 1, po1 if j == 2 else po2)
                proj_moves(j, ps)
                if j == 2:
                    po2 = attn(2)
                else:
                    po3 = attn(3)
            finalize(3, po3)

    nc.compile()
    return nc


def _pv(nc, po, Vn, pe_info, pos, npairs):
    pe, info = pe_info
    for half, (c, f0, off) in enumerate(info):
        nc.tensor.matmul(po[0:HS + 1, f0:QB], Vn[:, c, 0:HS + 1],
                         pe[:, f0 + off:off + QB],
                         start=(pos == 0 and half == 0),
                         stop=(pos == npairs - 1 and half == 1))


_RUNNERS = {}


def _get_runner(mode=None):
    mode = mode or MM_MODE
    if mode not in _RUNNERS:
        _RUNNERS[mode] = build(mode)
    return _RUNNERS[mode]


def _host_dt(mode=None):
    if (mode or MM_MODE) == "bf16":
        import ml_dtypes
        return ml_dtypes.bfloat16
    return np.float32


def make_in_maps(x, wq_w, wq_b, wk_w, wk_b, wv_w, wv_b, mode=None):
    hd = _host_dt(mode)
    x = np.asarray(x, np.float32)
    wqk = np.concatenate([np.asarray(wq_w, np.float32),
                          np.asarray(wk_w, np.float32)], axis=0)  # [2HS, D]
    # SBUF layout [p, dc, m]: value = W[m, dc*128 + p]
    wqkB = np.ascontiguousarray(
        wqk.T.reshape(ND, 128, 2 * HS).transpose(1, 0, 2)).astype(hd)
    wvB = np.ascontiguousarray(
        np.asarray(wv_w, np.float32).T.reshape(ND, 128, HS)
        .transpose(1, 0, 2)).astype(hd)
    qb = np.ascontiguousarray(np.asarray(wq_b, np.float32)[:, None])
    vb = np.ascontiguousarray(np.asarray(wv_b, np.float32)[:, None])
    in_maps = []
    for b in range(B):
        in_maps.append({
            "xT": np.ascontiguousarray(x[b].T).astype(hd),
            "wqkB": wqkB, "wvB": wvB, "qb": qb, "vb": vb,
        })
    return in_maps


def assemble(res):
    """Device out[p, j*4+tt, h] -> full [B, T, HS] (t = (j*4+tt)*128 + p)."""
    outs = []
    for b in range(B):
        o = np.asarray(res.results[b]["out"], np.float32)     # [128, 16, HS]
        outs.append(o.transpose(1, 0, 2).reshape(T, HS))
    return np.stack(outs, axis=0)


def run(in_maps, trace=False, tmpdir=None, mode=None):
    nc = _get_runner(mode)
    return run_bass_kernel_spmd(nc, in_maps, core_ids=list(range(NCORES)),
                                trace=trace, tmpdir=tmpdir)


def _canary_ok(out, x, wq_w, wq_b, wk_w, wk_b, wv_w, wv_b):
    """Cheap exact check of causal rows t=0,1 (closed-form, tiny host cost).

    Catches catastrophic HW-mode failures (zeros/garbage) while passing
    reduced-precision rounding. Row 0 attends only key 0 -> out = v[0];
    row 1 is a two-term softmax.
    """
    x2 = np.asarray(x, np.float32)[:, 0:2, :].astype(np.float64)      # [B,2,D]
    q = x2 @ np.asarray(wq_w, np.float64).T + np.asarray(wq_b, np.float64)
    k = x2 @ np.asarray(wk_w, np.float64).T + np.asarray(wk_b, np.float64)
    v = x2 @ np.asarray(wv_w, np.float64).T + np.asarray(wv_b, np.float64)
    exp0 = v[:, 0, :]                                                 # [B,HS]
    s = np.einsum("bh,bsh->bs", q[:, 1, :], k) / np.sqrt(HS)          # [B,2]
    w = np.exp(s - s.max(-1, keepdims=True))
    w = w / w.sum(-1, keepdims=True)
    exp1 = np.einsum("bs,bsh->bh", w, v)
    got = np.stack([out[:, 0, :], out[:, 1, :]], axis=1)
    want = np.stack([exp0, exp1], axis=1)
    rel = np.abs(got - want) / max(np.abs(want).max(), 1e-6)
    return np.isfinite(out).all() and rel.max() < 3e-2


def kernel(x, wq_w, wq_b, wk_w, wk_b, wv_w, wv_b):
    args = (x, wq_w, wq_b, wk_w, wk_b, wv_w, wv_b)
    res = run(make_in_maps(*args, mode=MM_MODE), mode=MM_MODE)
    out = assemble(res)
    if MM_MODE != FALLBACK_MODE and not _canary_ok(out, *args):
        # fast matmul mode produced bad numerics on this HW; fall back to
        # the plain-fp32 kernel
        res = run(make_in_maps(*args, mode=FALLBACK_MODE), mode=FALLBACK_MODE)
        out = assemble(res)
    return out
